# revision 26
# baseline (speedup 1.0000x reference)
"""Trainium2 Bass kernel for BasicRecurrentEntityEncoder.

Data-parallel over the batch (paragraph) dim: 8 cores x 8 paragraphs.
Per core, everything lives in a "columns" layout [d=128 partitions, n=160
free] where n = b_local*20 + k (8 paragraphs x 20 entity slots).

Phase A: embedding lookup via SWDGE dma_gather (Q7 ucode path; the Pool
engine costs ~8.5ns per gather index, so index count is the currency).
int16 gather indices address only 32768 rows, so the 50002-row padded
table is stored as 25001 PAIR-rows of 256 floats; each token gathers its
pair (idx = token>>1, 1KB payload) and the correct half is selected on the
vector engine with host-precomputed parity masks. Tokens stream s-major in
128-row slots of 6 whole sentences + 8 zero-pad rows; one matmul per slot
(lhsT = selected rows * pos_mask weights, rhs = 6-col sentence one-hot)
writes per-sentence sums into PSUM -> enc [128, 512], copied out chunk by
chunk. Gather calls are capped at 1024 indices (the Q7 per-call limit).

Phase B: 64 serial recurrence steps, EMISSION-INTERLEAVED with Phase A's
chunks (the engines are in-order, so a step's ops must be emitted as soon
as its data deps are emitted or it queues behind all of Phase A).
Off-critical-path precompute, chunked behind enc progress:
  zk[s]  = colsum(keysT * e_s) + mask_bias    (gate preact base, replicated)
  HTb[s] = (keys@V)^T + broadcast((e_s@W)^T)  (h_tilda preact base)
Per step, bases are DVE-written into PSUM (no matmul), and the critical
chain is:
  he=h*e [bf16 out] -> colsum (bf16 matmul accumulated onto zk base) ->
  Ex=exp(-z) [ACT] -> P=min(Ex,1e30)+1 -> gate=recip_approx(P) ->
  Tg=gate*HT -> yv=h+Tg -> SQ=yv*yv [bf16 out] -> ss=colsum (bf16 matmul)
  -> ln(ss+eps) [ACT] -> exp(-0.5*) [ACT] -> h_new = yv*rs
with HT=relu(U@h + base) joining from the side. Only the natural_log_exp
ACT table set is used -> no table switches. min(Ex,1e30) absorbs exp
overflow (incl. the -1e5 mask bias: masked sentences get gate ~ 1e-30 so
h passes through; the re-normalization is a no-op on unit/zero rows).
"""

import numpy as np

B, S, L, K, D, VOC = 64, 64, 20, 20, 128, 50000
NCORES = 8
BL = B // NCORES          # paragraphs per core = 8
N = BL * K                # recurrence columns = 160
G = S * BL                # sentence slots per core = 512
NPAIR = VOC // 2 + 1      # pair rows = 25001 (last pair all-zero)
SPS = 6                   # sentences per 128-token slot (120 real + 8 pad)
NSLOT = (G + SPS - 1) // SPS          # 86 slots
NIDX = NSLOT * 128                    # 11008 gather indices
CPS = 8                   # slots per gather chunk (1024 idxs = Q7 cap)
CHUNKS = [CPS] * (NSLOT // CPS) + ([NSLOT % CPS] if NSLOT % CPS else [])
GATE_BIAS = -1.0e5        # mask bias on gate preactivation
EPS = 1e-12

_NC_CACHE = {}


def _finish(nc):
    from concourse.library_overlay import lower_extended_insts
    lower_extended_insts(nc)
    return nc


def _build_nc(mode="full"):
    import concourse.bass as bass
    import concourse.tile as tile
    from concourse import mybir
    from concourse.library_config import mlp

    f32 = mybir.dt.float32
    bf16 = mybir.dt.bfloat16
    i16 = mybir.dt.int16
    AF = mybir.ActivationFunctionType
    OP = mybir.AluOpType

    nc = bass.Bass()

    # cst columns: id|ones|U|V|W|keysT|posw|omap|eps
    C_ID, C_ONE, C_U, C_V, C_W = 0, 128, 256, 384, 512
    C_KT, C_PW, C_OM, C_EPS = 640, 640 + N, 640 + N + 128, 640 + N + 128 + SPS
    CW = C_EPS + 1
    d_emb = nc.declare_dram_parameter("emb2", [NPAIR, 2 * D], f32, isOutput=False)
    d_idx = nc.declare_dram_parameter("idx", [128, NIDX // 16], i16, isOutput=False)
    d_par = nc.declare_dram_parameter("par", [128, 2 * NSLOT], f32, isOutput=False)
    d_mb = nc.declare_dram_parameter("mb", [1, G], f32, isOutput=False)
    d_cst = nc.declare_dram_parameter("cst", [128, CW], f32, isOutput=False)
    d_h0 = nc.declare_dram_parameter("h0", [D, N], f32, isOutput=False)
    if mode == "encA":
        d_out = nc.declare_dram_parameter("out", [128, G], f32, isOutput=True)
    else:
        d_out = nc.declare_dram_parameter("out", [D, N], f32, isOutput=True)

    from contextlib import ExitStack
    with ExitStack() as ctx:
        tc = ctx.enter_context(tile.TileContext(nc))
        singles = ctx.enter_context(tc.tile_pool(name="singles", bufs=1))
        g_pool = ctx.enter_context(tc.tile_pool(name="gp", bufs=2))
        m_pool = ctx.enter_context(tc.tile_pool(name="mp", bufs=2))
        wt_pool = ctx.enter_context(tc.tile_pool(name="wt", bufs=2))
        ke_pool = ctx.enter_context(tc.tile_pool(name="ke", bufs=2))
        zkf_pool = ctx.enter_context(tc.tile_pool(name="zkf", bufs=2))
        htf_pool = ctx.enter_context(tc.tile_pool(name="htf", bufs=2))
        hpool = ctx.enter_context(tc.tile_pool(name="hpool", bufs=2))
        step_sb = ctx.enter_context(tc.tile_pool(name="step_sb", bufs=2))
        p_enc = ctx.enter_context(tc.tile_pool(name="p_enc", bufs=1, space="PSUM"))
        p_pre = ctx.enter_context(tc.tile_pool(name="p_pre", bufs=1, space="PSUM"))
        p_g = ctx.enter_context(tc.tile_pool(name="p_g", bufs=2, space="PSUM"))
        p_ht = ctx.enter_context(tc.tile_pool(name="p_ht", bufs=2, space="PSUM"))
        p_ss = ctx.enter_context(tc.tile_pool(name="p_ss", bufs=2, space="PSUM"))

        # ---- inputs into SBUF ----
        idx_sb = singles.tile([128, NIDX // 16], i16)
        nc.sync.dma_start(out=idx_sb[:, :], in_=d_idx[:, :])
        par_sb = singles.tile([128, 2 * NSLOT], f32)
        nc.sync.dma_start(out=par_sb[:, :], in_=d_par[:, :])
        mb_sb = singles.tile([1, G], f32)
        nc.sync.dma_start(out=mb_sb[:, :], in_=d_mb[:, :])
        cst_sb = singles.tile([128, CW], f32)
        nc.sync.dma_start(out=cst_sb[:, :], in_=d_cst[:, :])
        ones_sb = cst_sb[:, C_ONE:C_ONE + 128]
        U_sb = cst_sb[:, C_U:C_U + 128]
        V_sb = cst_sb[:, C_V:C_V + 128]
        W_sb = cst_sb[:, C_W:C_W + 128]
        keysT_sb = cst_sb[:, C_KT:C_KT + N]
        posw_sb = cst_sb[:, C_PW:C_PW + 128]
        omap_sb = cst_sb[:, C_OM:C_OM + SPS]
        eps_sb = cst_sb[:, C_EPS:C_EPS + 1]

        # warmups: one tiny op per engine/lane so real instructions need at
        # most one semaphore wait after _legalize_waits
        warm = singles.tile([1, 4], f32)
        nc.vector.tensor_copy(out=warm[0:1, 0:1], in_=cst_sb[0:1, 0:1])
        nc.vector.tensor_copy(out=warm[0:1, 1:2], in_=mb_sb[0:1, 0:1])
        nc.scalar.copy(out=warm[0:1, 2:3], in_=cst_sb[0:1, 0:1])

        onesb_sb = singles.tile([128, 128], bf16)   # bf16 all-ones lhsT
        nc.vector.tensor_copy(out=onesb_sb[:, :], in_=ones_sb)

        enc_sb = singles.tile([128, G], f32)      # encoded sents (d, s*8+b)
        KV_sb = singles.tile([128, N], f32)       # (keys@V)^T
        HTb_sb = singles.tile([128, S * N], f32)  # h_tilda base
        zk_sb = singles.tile([1, S * N], f32)     # gate base
        omap0_sb = singles.tile([128, NSLOT * SPS], f32)  # parity-masked maps
        omap1_sb = singles.tile([128, NSLOT * SPS], f32)
        psum_enc = p_enc.tile([128, G], f32)

        nc.gpsimd.load_library(mlp)

        id_sb = cst_sb[:, C_ID:C_ID + 128]
        # parity-masked sentence maps: omap0/1[:, j*6+q] = omap[:, q]*par{0,1}[:, j]
        om_b = bass.AP(tensor=omap_sb.tensor, offset=omap_sb.offset,
                       ap=[omap_sb.ap[0], [0, NSLOT], [1, SPS]])
        pr0 = par_sb[:, 0:NSLOT]
        pr0_b = bass.AP(tensor=pr0.tensor, offset=pr0.offset,
                        ap=[pr0.ap[0], [1, NSLOT], [0, SPS]])
        nc.vector.tensor_tensor(
            out=omap0_sb[:, :].rearrange("p (a b) -> p a b", a=NSLOT),
            in0=om_b, in1=pr0_b, op=OP.mult)
        pr1 = par_sb[:, NSLOT:2 * NSLOT]
        pr1_b = bass.AP(tensor=pr1.tensor, offset=pr1.offset,
                        ap=[pr1.ap[0], [1, NSLOT], [0, SPS]])
        nc.vector.tensor_tensor(
            out=omap1_sb[:, :].rearrange("p (a b) -> p a b", a=NSLOT),
            in0=om_b, in1=pr1_b, op=OP.mult)

        if mode != "encA":
            # (keys@V)^T -- depends only on cst
            ps = p_pre.tile([128, 320], f32, tag="pre")
            nc.tensor.matmul(out=ps[:, 0:N], lhsT=V_sb, rhs=keysT_sb,
                             start=True, stop=True)
            nc.scalar.copy(out=KV_sb[:, :], in_=ps[:, 0:N])

        def emit_pre_granule(g):
            """zk + h_tilda bases for steps [4g, 4g+4).
            Uses enc cols [32g, 32g+32)."""
            # gate base: colsum(keysT * e_s) + mask bias
            ke = ke_pool.tile([128, 4 * N], f32, tag="ke")
            kt_b = bass.AP(tensor=keysT_sb.tensor, offset=keysT_sb.offset,
                           ap=[keysT_sb.ap[0], [0, 4], [K, BL], [1, K]])
            e4 = enc_sb[:, 32 * g:32 * (g + 1)]
            e4_b = bass.AP(tensor=e4.tensor, offset=e4.offset,
                           ap=[e4.ap[0], [BL, 4], [1, BL], [0, K]])
            nc.vector.tensor_tensor(
                out=ke[:, :].rearrange("p (s b k) -> p s b k", s=4, b=BL),
                in0=kt_b, in1=e4_b, op=OP.mult)
            for h in range(2):
                ps = p_pre.tile([128, 320], f32, tag="pre")
                nc.tensor.matmul(out=ps[:, 0:320], lhsT=ones_sb,
                                 rhs=ke[:, 320 * h:320 * (h + 1)],
                                 start=True, stop=True)
                mb2 = mb_sb[0:1, 32 * g + 16 * h:32 * g + 16 * (h + 1)]
                mb_b = bass.AP(tensor=mb2.tensor, offset=mb2.offset,
                               ap=[mb2.ap[0], [BL, 2], [1, BL], [0, K]])
                nc.vector.tensor_tensor(
                    out=zk_sb[0:1, 4 * g * N + 320 * h:
                              4 * g * N + 320 * (h + 1)].rearrange(
                        "p (s b k) -> p s b k", s=2, b=BL),
                    in0=ps[0:1, 0:320].rearrange("p (s b k) -> p s b k",
                                                 s=2, b=BL),
                    in1=mb_b, op=OP.add)
            o0 = 4 * g * N
            # h_tilda base: KV + broadcast((e@W)^T)
            ps = p_pre.tile([128, 320], f32, tag="pre")
            nc.tensor.matmul(out=ps[:, 0:32], lhsT=W_sb,
                             rhs=enc_sb[:, 32 * g:32 * (g + 1)],
                             start=True, stop=True)
            kv_b = bass.AP(tensor=KV_sb.tensor, offset=KV_sb.offset,
                           ap=[KV_sb.ap[0], [0, 4], [K, BL], [1, K]])
            ew_b = bass.AP(tensor=ps.tensor, offset=ps.offset,
                           ap=[ps.ap[0], [BL, 4], [1, BL], [0, K]])
            nc.vector.tensor_tensor(
                out=HTb_sb[:, o0:o0 + 4 * N].rearrange(
                    "p (s b k) -> p s b k", s=4, b=BL),
                in0=kv_b, in1=ew_b, op=OP.add)

        # ---- Phase B step emitter (interleaved with Phase A chunks) ----
        state = {"h": None, "pg": None, "pht": None, "em": None}

        def emit_e_mat(s):
            """Materialize broadcast e for step s (off-path; fills a DVE
            stall so the on-path he multiply is a contiguous fast op)."""
            e8 = enc_sb[:, s * BL:(s + 1) * BL]
            e_rep = bass.AP(tensor=e8.tensor, offset=e8.offset,
                            ap=[e8.ap[0], e8.ap[1], [0, K]])
            em = step_sb.tile([D, N], f32, tag="em")
            nc.vector.tensor_copy(
                out=em[:, :].rearrange("p (a b) -> p a b", a=BL),
                in_=e_rep)
            return em

        def emit_base_writes(s):
            """Matmul the step-s preact bases into fresh PSUM banks
            (off the critical path; TE absorbs these during chain stalls)."""
            pg = p_g.tile([D, N], f32, tag="g")
            nc.tensor.matmul(out=pg[:, :], lhsT=ones_sb[0:1, :],
                             rhs=zk_sb[0:1, s * N:(s + 1) * N],
                             start=True, stop=False)
            pht = p_ht.tile([D, N], f32, tag="ht")
            nc.tensor.matmul(out=pht[:, :], lhsT=id_sb,
                             rhs=HTb_sb[:, s * N:(s + 1) * N],
                             start=True, stop=False)
            return pg, pht

        def emit_step(s):
            if s == 0:
                h_prev = hpool.tile([D, N], f32, tag="h")
                nc.sync.dma_start(out=h_prev[:, :], in_=d_h0[:, :])
                state["h"] = h_prev
                state["pg"], state["pht"] = emit_base_writes(0)
                state["em"] = emit_e_mat(0)
            h_prev = state["h"]
            psum_g, psum_ht = state["pg"], state["pht"]
            e_mat = state["em"]

            he = step_sb.tile([D, N], bf16, tag="he")
            nc.vector.tensor_tensor(out=he[:, :], in0=h_prev[:, :],
                                    in1=e_mat[:, :], op=OP.mult)
            nc.tensor.matmul(out=psum_g[:, :], lhsT=onesb_sb, rhs=he[:, :],
                             start=False, stop=True)
            nc.tensor.matmul(out=psum_ht[:, :], lhsT=U_sb, rhs=h_prev[:, :],
                             start=False, stop=True)
            # next step's bases follow on TE before this step's ss colsum
            if s + 1 < S:
                state["pg"], state["pht"] = emit_base_writes(s + 1)

            HT = step_sb.tile([D, N], f32, tag="HT")
            nc.vector.tensor_scalar(out=HT[:, :], in0=psum_ht[:, :],
                                    scalar1=0.0, scalar2=0.0,
                                    op0=OP.max, op1=OP.add)
            Ex = step_sb.tile([D, N], f32, tag="Ex")
            nc.scalar.activation(Ex[:, :], psum_g[:, :], AF.Exp,
                                 bias=0.0, scale=-1.0)

            P = step_sb.tile([D, N], f32, tag="P")
            nc.vector.tensor_scalar(out=P[:, :], in0=Ex[:, :],
                                    scalar1=1.0e30, scalar2=1.0,
                                    op0=OP.min, op1=OP.add)
            gate = step_sb.tile([D, N], f32, tag="gate")
            nc.vector.reciprocal_approx_fast(out=gate[:, :], in_=P[:, :])
            Tg = step_sb.tile([D, N], f32, tag="Tg")
            nc.vector.tensor_tensor(out=Tg[:, :], in0=gate[:, :],
                                    in1=HT[:, :], op=OP.mult)
            yv = step_sb.tile([D, N], f32, tag="yv")
            nc.vector.tensor_tensor(out=yv[:, :], in0=h_prev[:, :],
                                    in1=Tg[:, :], op=OP.add)
            SQ = step_sb.tile([D, N], bf16, tag="SQ")
            nc.vector.tensor_tensor(out=SQ[:, :], in0=yv[:, :],
                                    in1=yv[:, :], op=OP.mult)
            if s + 1 < S:
                state["em"] = emit_e_mat(s + 1)
            psum_ss = p_ss.tile([D, N], f32, tag="ss")
            nc.tensor.matmul(out=psum_ss[:, :], lhsT=onesb_sb, rhs=SQ[:, :],
                             start=True, stop=True)

            Lg = step_sb.tile([D, N], f32, tag="Lg")
            nc.scalar.activation(Lg[:, :], psum_ss[:, :], AF.Ln, bias=eps_sb)
            RS = step_sb.tile([D, N], f32, tag="RS")
            nc.scalar.activation(RS[:, :], Lg[:, :], AF.Exp,
                                 bias=0.0, scale=-0.5)
            h_new = hpool.tile([D, N], f32, tag="h")
            nc.vector.tensor_tensor(out=h_new[:, :], in0=yv[:, :],
                                    in1=RS[:, :], op=OP.mult)
            state["h"] = h_new

        # ---- Phase A chunks with interleaved precompute + steps ----
        # Steps are emitted ONE CHUNK BEHIND the phase-A stream so chunk
        # ops sit ahead of step ops in each engine queue and execute during
        # chain stalls instead of behind them.
        gran_done = 0
        steps_done = 0
        gran_prev = 0          # granules emitted as of one chunk ago
        j0 = 0
        for ci, nsl in enumerate(CHUNKS):
            nidx = nsl * 128
            gt = g_pool.tile([128, CPS * 2 * D], f32, tag="gt")
            nc.gpsimd.dma_gather(
                out_ap=gt[:, 0:nsl * 2 * D].rearrange("p (a b) -> p a b", a=nsl),
                in_ap=d_emb[:, :],
                idxs_ap=idx_sb[:, j0 * 8:(j0 + nsl) * 8],
                num_idxs=nidx, num_idxs_reg=nidx, elem_size=2 * D,
                queue_num=0,
            )
            # select the correct pair-half on DVE, then position-weight;
            # single matmul per slot keeps the gather-paced TE stream short
            gt3 = gt[:, 0:nsl * 2 * D].rearrange("p (a b) -> p a b", a=nsl)
            g_ev = bass.AP(tensor=gt3.tensor, offset=gt3.offset,
                           ap=[gt3.ap[0], gt3.ap[1], [1, D]])
            g_od = bass.AP(tensor=gt3.tensor, offset=gt3.offset + D,
                           ap=[gt3.ap[0], gt3.ap[1], [1, D]])
            p0 = par_sb[:, j0:j0 + nsl]
            p0_b = bass.AP(tensor=p0.tensor, offset=p0.offset,
                           ap=[p0.ap[0], p0.ap[1], [0, D]])
            p1 = par_sb[:, NSLOT + j0:NSLOT + j0 + nsl]
            p1_b = bass.AP(tensor=p1.tensor, offset=p1.offset,
                           ap=[p1.ap[0], p1.ap[1], [0, D]])
            m0 = m_pool.tile([128, CPS * D], f32, tag="m0")
            m0r = m0[:, 0:nsl * D].rearrange("p (a b) -> p a b", a=nsl)
            nc.vector.tensor_tensor(out=m0r, in0=g_ev, in1=p0_b, op=OP.mult)
            m1 = m_pool.tile([128, CPS * D], f32, tag="m1")
            m1r = m1[:, 0:nsl * D].rearrange("p (a b) -> p a b", a=nsl)
            nc.vector.tensor_tensor(out=m1r, in0=g_od, in1=p1_b, op=OP.mult)
            gs = m_pool.tile([128, CPS * D], f32, tag="gs")
            nc.vector.tensor_tensor(out=gs[:, 0:nsl * D], in0=m0[:, 0:nsl * D],
                                    in1=m1[:, 0:nsl * D], op=OP.add)
            wt = wt_pool.tile([128, CPS * D], f32, tag="wt")
            pw_b = bass.AP(tensor=posw_sb.tensor, offset=posw_sb.offset,
                           ap=[posw_sb.ap[0], [0, nsl], [1, 128]])
            nc.vector.tensor_tensor(
                out=wt[:, 0:nsl * D].rearrange("p (a b) -> p a b", a=nsl),
                in0=gs[:, 0:nsl * D].rearrange("p (a b) -> p a b", a=nsl),
                in1=pw_b, op=OP.mult)
            for jj in range(nsl):
                j = j0 + jj
                g0, g1 = SPS * j, min(SPS * (j + 1), G)
                nc.tensor.matmul(
                    out=psum_enc[:, g0:g1],
                    lhsT=wt[:, jj * D:(jj + 1) * D],
                    rhs=omap_sb[:, 0:g1 - g0],
                    start=True, stop=True,
                )
            j0 += nsl
            e0, e1 = SPS * (j0 - nsl), min(SPS * j0, G)
            nc.scalar.copy(out=enc_sb[:, e0:e1], in_=psum_enc[:, e0:e1])

            cols_done = min(SPS * j0, G)
            if mode != "encA":
                while gran_done < S // 4 and 32 * (gran_done + 1) <= cols_done:
                    emit_pre_granule(gran_done)
                    gran_done += 1
                while steps_done < S:
                    s = steps_done
                    need = min(s + 1, S - 1)   # step s pre-writes s+1 bases
                    if need // 4 >= gran_prev:
                        break
                    emit_step(s)
                    steps_done += 1
                gran_prev = gran_done

        if mode == "encA":
            nc.sync.dma_start(out=d_out[:, :], in_=enc_sb[:, :])
            return _finish(nc)

        while steps_done < S:
            emit_step(steps_done)
            steps_done += 1

        nc.sync.dma_start(out=d_out[:, :], in_=state["h"][:, :])

    return _finish(nc)


def _legalize_waits(bir_json: bytes) -> bytes:
    """Walrus codegen allows at most ONE sem-wait per instruction; Tile's sem
    assigner emits several. Hoist all but the last wait onto EventSemaphore
    carrier instructions inserted just before the offender (same engine, so
    in-order execution preserves semantics exactly)."""
    import orjson
    bir = orjson.loads(bir_json)
    n_new = 0
    for fn in bir.get("functions", []):
        for bb in fn.get("blocks", []):
            out = []
            for inst in bb.get("instructions", []):
                si = inst.get("sync_info") or {}
                ow = si.get("on_wait") or []
                if len(ow) > 1:
                    for w in ow[:-1]:
                        n_new += 1
                        out.append({
                            "debug": inst.get("debug", 0),
                            "engine": inst["engine"],
                            "ins": [], "outs": [],
                            "name": f"waitfix_{n_new}_{inst.get('name','')}",
                            "opcode": "EventSemaphore",
                            "sync_info": {"on_update": [], "on_wait": [w]},
                        })
                    si["on_wait"] = [ow[-1]]
                    inst["sync_info"] = si
                out.append(inst)
            bb["instructions"] = out
    return orjson.dumps(bir)


def _install_compile_hook():
    import concourse.bass2jax as b2j
    if getattr(b2j, "_waitfix_installed", False):
        return
    orig = b2j.compile_bir_kernel

    def patched(bir_json, tmpdir, neff_name="file.neff"):
        return orig(_legalize_waits(bir_json), tmpdir, neff_name)

    b2j.compile_bir_kernel = patched
    b2j._waitfix_installed = True


def get_nc(mode="full"):
    if mode not in _NC_CACHE:
        _NC_CACHE[mode] = _build_nc(mode)
    return _NC_CACHE[mode]


def make_shared_consts(embedding_matrix, pos_mask):
    f32 = np.float32
    emb = np.asarray(embedding_matrix, dtype=f32)
    emb2 = np.vstack([emb, np.zeros((2, D), dtype=f32)]).reshape(NPAIR, 2 * D)

    pm = np.asarray(pos_mask, dtype=f32)
    posw = np.zeros((128, D), dtype=f32)
    for r in range(SPS * L):
        posw[r] = pm[r % L]
    omap = np.zeros((128, SPS), dtype=f32)
    for r in range(SPS * L):
        omap[r, r // L] = 1.0

    C_PW, C_OM, C_EPS = 640 + N, 640 + N + 128, 640 + N + 128 + SPS
    CW = C_EPS + 1
    cst = np.zeros((128, CW), dtype=f32)
    cst[:, 0:128] = np.eye(128, dtype=f32)
    cst[:, 128:256] = 1.0
    cst[:, C_PW:C_PW + 128] = posw
    cst[:, C_OM:C_OM + SPS] = omap
    cst[:, C_EPS] = EPS
    return np.ascontiguousarray(emb2), cst


def make_inputs_for_core(c, prgrph, prgrph_mask, keys, U, V, W, emb2, cst):
    f32 = np.float32
    bsl = slice(c * BL, (c + 1) * BL)
    pr = np.asarray(prgrph[bsl]).astype(np.int64)      # [BL, S, L]
    mk = np.asarray(prgrph_mask[bsl]).astype(bool)     # [BL, S, L]
    ky = np.asarray(keys[bsl], dtype=f32)              # [BL, K, D]

    # token stream: g-major (g = s*BL + b), l within sentence; padded to
    # NSLOT slots of 120 real + 8 pad rows
    t = np.where(mk, pr, VOC).transpose(1, 0, 2).reshape(-1)   # [10240]
    tok = np.full((NSLOT, 128), VOC, dtype=np.int64)           # [86, 128]
    tpad = np.full(NSLOT * SPS * L, VOC, dtype=np.int64)
    tpad[:t.size] = t
    tok[:, :SPS * L] = tpad.reshape(NSLOT, SPS * L)
    flat = tok.reshape(-1)                                      # i = j*128+r
    idx = (flat >> 1).astype(np.int16)
    # parity masks [128 r, NSLOT j]: par0 selects even row, par1 odd
    parity = (flat & 1).astype(f32).reshape(NSLOT, 128).T       # [128, NSLOT]
    par = np.concatenate([1.0 - parity, parity], axis=1)        # [128, 2*NSLOT]

    # idx element i -> partition i%16, col i//16; replicated over 8 stripes
    t16 = np.ascontiguousarray(idx.reshape(NIDX // 16, 16).T)
    idx_tile = np.ascontiguousarray(np.tile(t16, (8, 1)))

    keysT = np.ascontiguousarray(ky.transpose(2, 0, 1).reshape(D, N))

    m_s = mk[:, :, 0].astype(f32)                       # [BL, S]
    mb = (m_s.T.reshape(-1) - 1.0) * (-GATE_BIAS)       # [G]: 0 or GATE_BIAS

    cst_c = cst.copy()
    cst_c[:, 256:384] = np.asarray(U, dtype=f32)
    cst_c[:, 384:512] = np.asarray(V, dtype=f32)
    cst_c[:, 512:640] = np.asarray(W, dtype=f32)
    cst_c[:, 640:640 + N] = keysT
    return {
        "emb2": emb2,
        "idx": idx_tile,
        "par": np.ascontiguousarray(par, dtype=f32),
        "mb": np.ascontiguousarray(mb.reshape(1, G), dtype=f32),
        "cst": np.ascontiguousarray(cst_c),
        "h0": np.zeros((D, N), dtype=f32),
    }


def kernel(prgrph, prgrph_mask, keys, embedding_matrix, pos_mask, U, V, W):
    from concourse.bass_utils import run_bass_kernel_spmd
    _install_compile_hook()

    emb2, cst = make_shared_consts(embedding_matrix, pos_mask)
    in_maps = [
        make_inputs_for_core(c, prgrph, prgrph_mask, keys, U, V, W, emb2, cst)
        for c in range(NCORES)
    ]
    nc = get_nc()
    res = run_bass_kernel_spmd(nc, in_maps, core_ids=list(range(NCORES)))
    outs = []
    for c in range(NCORES):
        o = np.asarray(res.results[c]["out"])        # [D, N]
        outs.append(o.T.reshape(BL, K, D))
    return np.concatenate(outs, axis=0).astype(np.float32)
